# revision 20
# baseline (speedup 1.0000x reference)
# Self-contained Bass/Trainium2 kernel for the BipartiteGNN problem.
# kernel(**inputs) takes FULL inputs, shards across 8 NeuronCores internally,
# and returns the FULL [N_P] output.
#
# Sharding: children + edges sharded by src across the 8 cores (hc never
# replicated); hp replicated via redundant node compute; parent aggregation via
# one AllReduce (layer 1) and one ReduceScatter (layer 2). The per-edge
# "predicted" MLP depends only on dst, so it is computed once per parent node
# (pred table) and gathered per edge. Segment sums are computed per 128-edge
# bin (host-side sort + bin packing so no parent/child run straddles a bin)
# with an is_equal selection matmul, then scattered with collision-safe writes.
import os

import numpy as np

import concourse.bass as bass
import concourse.mybir as mybir
import concourse.tile as tile
from concourse import bacc
from concourse.masks import make_identity
from concourse.bass_utils import run_bass_kernel_spmd

F32 = mybir.dt.float32
BF16 = mybir.dt.bfloat16
I32 = mybir.dt.int32
AF = mybir.ActivationFunctionType
OP = mybir.AluOpType

N_P, N_C, E = 100000, 400000, 800000
D_P, D_C, H = 32, 32, 64
NCORES = 8
CSH = N_C // NCORES          # child shard size per core (50000)
PSH = N_P // NCORES          # parent shard size per core (12500)
EPS = 1e-5
G = 8                        # bins per superstep in edge phases
PART1_ROWS = 102400          # padded parent scatter buffer (128*800)
AGGRC_ROWS = 51200           # padded child scatter buffer (128*400)

DEBUG = bool(int(os.environ.get("GNN_DEBUG", "0")))


# ----------------------------------------------------------------------------
# Host-side preprocessing
# ----------------------------------------------------------------------------

def _binpack(sorted_vals, n_edges):
    """Pack positions of a sorted array into bins of 128 such that no value's
    run straddles a bin boundary. Returns [nbins, 128], -1 = padding."""
    if n_edges == 0:
        return np.full((0, 128), -1, np.int64)
    change = np.nonzero(np.diff(sorted_vals))[0] + 1
    starts = np.concatenate(([0], change))
    ends = np.concatenate((change, [n_edges]))
    lens = ends - starts
    assert lens.max() <= 128, f"run of length {lens.max()} > 128"
    bins = []
    cur = []
    for s, l in zip(starts.tolist(), lens.tolist()):
        if len(cur) + l > 128:
            bins.append(cur)
            cur = []
        cur.extend(range(s, s + l))
    if cur:
        bins.append(cur)
    out = np.full((len(bins), 128), -1, np.int64)
    for i, b in enumerate(bins):
        out[i, : len(b)] = b
    return out


def _take(arr_sorted, bins, pad):
    out = np.full(bins.shape, pad, arr_sorted.dtype)
    m = bins >= 0
    out[m] = arr_sorted[bins[m]]
    return out


def _prep_edges(src, dst):
    """Per-core edge streams + metadata as [128, nbins] arrays (DMA-ready)."""
    src = np.asarray(src, np.int64)
    dst = np.asarray(dst, np.int64)
    cnt_dst = np.bincount(dst, minlength=N_P)
    cnt_src = np.bincount(src, minlength=N_C)
    w_dst = (1.0 / np.maximum(cnt_dst, 1)).astype(np.float32)
    w_src = (1.0 / np.maximum(cnt_src, 1)).astype(np.float32)

    core_of = src // CSH
    per_core = []
    for k in range(NCORES):
        eids = np.nonzero(core_of == k)[0]
        s_k, d_k = src[eids], dst[eids]
        # msg stream: sorted by dst
        o = np.argsort(d_k, kind="stable")
        bins_m = _binpack(d_k[o], len(eids))
        dst_m = _take(d_k[o], bins_m, -1)
        src_m = _take(s_k[o], bins_m, 0)
        pad_m = bins_m < 0
        g_pred = np.where(pad_m, 0, dst_m).astype(np.int32)
        g_xj = np.where(pad_m, 0, (src_m - k * CSH) + N_P).astype(np.int32)
        dstf = np.where(pad_m, -1.0, dst_m.astype(np.float64)).astype(np.float32)
        wv = np.where(pad_m, 0.0, w_dst[np.where(dst_m < 0, 0, dst_m)]).astype(np.float32)
        scat = np.where(pad_m, N_P, dst_m).astype(np.int32)
        # child stream: sorted by src
        o2 = np.argsort(s_k, kind="stable")
        bins_c = _binpack(s_k[o2], len(eids))
        src_c = _take(s_k[o2], bins_c, -1)
        dst_c = _take(d_k[o2], bins_c, 0)
        pad_c = bins_c < 0
        c_gidx = np.where(pad_c, 0, dst_c).astype(np.int32)
        srcf = np.where(pad_c, -1.0, src_c.astype(np.float64)).astype(np.float32)
        wv2 = np.where(pad_c, 0.0, w_src[np.where(src_c < 0, 0, src_c)]).astype(np.float32)
        scat2 = np.where(pad_c, CSH, src_c - k * CSH).astype(np.int32)
        per_core.append(dict(g_pred=g_pred, g_xj=g_xj, dstf=dstf, wv=wv, scat=scat,
                             c_gidx=c_gidx, srcf=srcf, wv2=wv2, scat2=scat2))

    nb1 = max(d["g_pred"].shape[0] for d in per_core)
    nb1 = ((nb1 + G - 1) // G) * G
    nb2 = max(d["c_gidx"].shape[0] for d in per_core)
    nb2 = ((nb2 + G - 1) // G) * G

    metas = []
    for d in per_core:
        def padto(a, nb, pad):
            out = np.full((nb, 128), pad, a.dtype)
            out[: a.shape[0]] = a
            return out
        g_pred = padto(d["g_pred"], nb1, np.int32(0))
        g_xj = padto(d["g_xj"], nb1, np.int32(N_P))
        dstf = padto(d["dstf"], nb1, np.float32(-1.0))
        wv = padto(d["wv"], nb1, np.float32(0.0))
        scat = padto(d["scat"], nb1, np.int32(N_P))
        c_g = padto(d["c_gidx"], nb2, np.int32(0))
        srcf = padto(d["srcf"], nb2, np.float32(-1.0))
        wv2 = padto(d["wv2"], nb2, np.float32(0.0))
        scat2 = padto(d["scat2"], nb2, np.int32(CSH))
        # interleave gather indices: per superstep, G pred cols then G xj cols
        gidx = np.empty((nb1 // G, 2 * G, 128), np.int32)
        gidx[:, :G] = g_pred.reshape(nb1 // G, G, 128)
        gidx[:, G:] = g_xj.reshape(nb1 // G, G, 128)
        gidx = gidx.reshape(2 * nb1, 128)
        metas.append(dict(
            m_gidx=np.ascontiguousarray(gidx.T),
            m_dstf=np.ascontiguousarray(dstf.T),
            m_w=np.ascontiguousarray(wv.T),
            m_scat=np.ascontiguousarray(scat.T),
            c_gidx=np.ascontiguousarray(c_g.T),
            c_srcf=np.ascontiguousarray(srcf.T),
            c_w=np.ascontiguousarray(wv2.T),
            c_scat=np.ascontiguousarray(scat2.T),
        ))
    return metas, nb1, nb2


def _fold_params(params):
    P = {}
    pe, ce = params["parent_enc"], params["child_enc"]
    P["wp1"] = np.asarray(pe["w1"], np.float32)
    P["bp1"] = np.asarray(pe["b1"], np.float32).reshape(H, 1)
    P["wp2"] = np.asarray(pe["w2"], np.float32)
    P["bp2"] = np.asarray(pe["b2"], np.float32).reshape(H, 1)
    P["wc1"] = np.asarray(ce["w1"], np.float32)
    P["bc1"] = np.asarray(ce["b1"], np.float32).reshape(H, 1)
    P["wc2"] = np.asarray(ce["w2"], np.float32)
    P["bc2"] = np.asarray(ce["b2"], np.float32).reshape(H, 1)
    for i, lay in enumerate(params["layers"]):
        P[f"l{i}_pw1"] = np.asarray(lay["pred_w1"], np.float32)
        P[f"l{i}_pb1"] = np.asarray(lay["pred_b1"], np.float32).reshape(H, 1)
        P[f"l{i}_pw2"] = np.asarray(lay["pred_w2"], np.float32)
        P[f"l{i}_pb2"] = np.asarray(lay["pred_b2"], np.float32).reshape(H, 1)
        # fold msg layernorm affine into the update matmul (exact when b==0)
        g = np.asarray(lay["msg_ln_g"], np.float32)
        b = np.asarray(lay["msg_ln_b"], np.float32)
        assert np.all(b == 0.0), "msg_ln_b folding requires b == 0"
        updw = np.asarray(lay["upd_w"], np.float32).copy()
        updw[H:] = g[:, None] * updw[H:]
        P[f"l{i}_updw"] = updw
        P[f"l{i}_updb"] = np.asarray(lay["upd_b"], np.float32).reshape(H, 1)
        P[f"l{i}_lnpg"] = np.asarray(lay["ln_p_g"], np.float32).reshape(1, H)
        P[f"l{i}_lnpb"] = np.asarray(lay["ln_p_b"], np.float32).reshape(1, H)
        if i == 0:
            P[f"l{i}_sagew"] = np.concatenate(
                [np.asarray(lay["sage_wl"], np.float32),
                 np.asarray(lay["sage_wr"], np.float32)], axis=0)
            P[f"l{i}_sageb"] = np.asarray(lay["sage_bl"], np.float32).reshape(H, 1)
            P[f"l{i}_lncg"] = np.asarray(lay["ln_c_g"], np.float32).reshape(1, H)
            P[f"l{i}_lncb"] = np.asarray(lay["ln_c_b"], np.float32).reshape(1, H)
    P["hw1"] = np.asarray(params["head_w1"], np.float32)
    P["hb1"] = np.asarray(params["head_b1"], np.float32).reshape(H // 2, 1)
    P["hw2"] = np.asarray(params["head_w2"], np.float32)
    P["hb2"] = np.asarray(params["head_b2"], np.float32).reshape(1, 1)
    return P


# ----------------------------------------------------------------------------
# Device program
# ----------------------------------------------------------------------------

def build_program(Pw, nb1, nb2, ln_identity):
    NS1, NS2 = nb1 // G, nb2 // G
    nc = bacc.Bacc("TRN2", target_bir_lowering=False, debug=False,
                   num_devices=NCORES)

    xpT = nc.dram_tensor("xpT", [D_P, N_P], F32, kind="ExternalInput")
    xcT = nc.dram_tensor("xcT", [D_C, CSH], F32, kind="ExternalInput")
    win = {}
    for name, arr in Pw.items():
        win[name] = nc.dram_tensor(name, list(arr.shape), F32, kind="ExternalInput")
    m_gidx = nc.dram_tensor("m_gidx", [128, 2 * nb1], I32, kind="ExternalInput")
    m_dstf = nc.dram_tensor("m_dstf", [128, nb1], F32, kind="ExternalInput")
    m_w = nc.dram_tensor("m_w", [128, nb1], F32, kind="ExternalInput")
    m_scat = nc.dram_tensor("m_scat", [128, nb1], I32, kind="ExternalInput")
    c_gidx = nc.dram_tensor("c_gidx", [128, nb2], I32, kind="ExternalInput")
    c_srcf = nc.dram_tensor("c_srcf", [128, nb2], F32, kind="ExternalInput")
    c_w = nc.dram_tensor("c_w", [128, nb2], F32, kind="ExternalInput")
    c_scat = nc.dram_tensor("c_scat", [128, nb2], I32, kind="ExternalInput")

    hp0 = nc.dram_tensor("hp0", [N_P, H], F32)
    tbl1 = nc.dram_tensor("tbl1", [N_P + CSH, H], F32)
    tbl2 = nc.dram_tensor("tbl2", [N_P + CSH, H], F32)
    hp1 = nc.dram_tensor("hp1", [N_P, H], F32)
    part1 = nc.dram_tensor("part1", [PART1_ROWS, H], BF16)
    part2 = nc.dram_tensor("part2", [PART1_ROWS, H], BF16)
    aggrc = nc.dram_tensor("aggrc", [AGGRC_ROWS, H], F32)
    aggr1 = nc.dram_tensor("aggr1", [N_P, H], BF16, addr_space="Shared")
    rs2 = nc.dram_tensor("rs2", [PSH, H], BF16)
    out = nc.dram_tensor("out", [1, PSH], F32, kind="ExternalOutput")

    with tile.TileContext(nc) as tc, \
         tc.tile_pool(name="consts", bufs=1) as consts, \
         tc.tile_pool(name="stats", bufs=4) as stats, \
         tc.tile_pool(name="work", bufs=3) as work, \
         tc.tile_pool(name="gathp", bufs=3) as gathp, \
         tc.tile_pool(name="fmp", bufs=2) as fmp, \
         tc.tile_pool(name="psum", bufs=1, space="PSUM") as psum:

        # PSUM discipline (8 banks total):
        #  tag "mm"  [64,512]   1 bank x3 slots
        #  tag "rm"  [128,4,64] 1 bank x1
        #  tag "sel" [128,8,128] 2 banks x1
        #  tag "dd"  [128,8,64] 1 bank x2
        def ps_mm():
            return psum.tile([H, 512], F32, tag="mm", bufs=2, name="psmm")
        def ps_rm():
            return psum.tile([128, 4, H], F32, tag="rm", bufs=1, name="psrm")
        def ps_sel():
            return psum.tile([128, G, 128], F32, tag="sel", bufs=1, name="pssel")
        def ps_cat():
            return psum.tile([128, 512], F32, tag="cat", bufs=1, name="pscat")
        def ps_dd():
            return psum.tile([128, G, H], F32, tag="dd", bufs=2, name="psdd")

        # ---------------- constants ----------------
        ident = consts.tile([128, 128], F32, tag="ident")
        make_identity(nc, ident[:])
        zeros = consts.tile([128, 3200], F32, tag="zeros")
        nc.vector.memset(zeros[:], 0.0)
        zerosb = consts.tile([128, 3200], BF16, tag="zerosb")
        nc.vector.memset(zerosb[:], 0.0)
        epsc = consts.tile([128, 1], F32, tag="epsc")
        nc.vector.memset(epsc[:], EPS)

        W = {}
        for name, arr in Pw.items():
            if arr.shape[0] == 2 * H:
                ta = consts.tile([H, H], F32, tag="w_" + name + "_a", name="wa")
                nc.gpsimd.dma_start(out=ta[:], in_=win[name][0:H, :])
                tb = consts.tile([H, H], F32, tag="w_" + name + "_b", name="wb")
                nc.gpsimd.dma_start(out=tb[:], in_=win[name][H:2 * H, :])
                W[name + "_a"], W[name + "_b"] = ta, tb
            else:
                t = consts.tile(list(arr.shape), F32, tag="w_" + name)
                nc.gpsimd.dma_start(out=t[:], in_=win[name][:])
                W[name] = t
        lnbc = {}
        for nm in ("l0_lnpg", "l0_lnpb", "l0_lncg", "l0_lncb", "l1_lnpg", "l1_lnpb"):
            if ln_identity.get(nm, False):
                lnbc[nm] = None
            else:
                t = consts.tile([128, H], F32, tag="bc_" + nm)
                nc.gpsimd.dma_start(out=t[:], in_=win[nm][0:1, :].partition_broadcast(128))
                lnbc[nm] = t

        sb_gidx = consts.tile([128, 2 * nb1], I32, tag="sb_gidx")
        nc.gpsimd.dma_start(out=sb_gidx[:], in_=m_gidx[:])
        sb_dstf = consts.tile([128, nb1], F32, tag="sb_dstf")
        nc.gpsimd.dma_start(out=sb_dstf[:], in_=m_dstf[:])
        sb_w = consts.tile([128, nb1], F32, tag="sb_w")
        nc.gpsimd.dma_start(out=sb_w[:], in_=m_w[:])
        sb_scat = consts.tile([128, nb1], I32, tag="sb_scat")
        nc.gpsimd.dma_start(out=sb_scat[:], in_=m_scat[:])
        sb_cgidx = consts.tile([128, nb2], I32, tag="sb_cgidx")
        nc.gpsimd.dma_start(out=sb_cgidx[:], in_=c_gidx[:])
        sb_csrcf = consts.tile([128, nb2], F32, tag="sb_csrcf")
        nc.gpsimd.dma_start(out=sb_csrcf[:], in_=c_srcf[:])
        sb_cw = consts.tile([128, nb2], F32, tag="sb_cw")
        nc.gpsimd.dma_start(out=sb_cw[:], in_=c_w[:])
        sb_cscat = consts.tile([128, nb2], I32, tag="sb_cscat")
        nc.gpsimd.dma_start(out=sb_cscat[:], in_=c_scat[:])

        def zero_fill(t, rows):
            zsrc = zeros if t.dtype == F32 else zerosb
            v = t[:].rearrange("(p b) d -> p (b d)", p=128)
            total = v.shape[1]
            for i in range(0, total, 3200):
                w = min(3200, total - i)
                nc.gpsimd.dma_start(out=v[:, i:i + w], in_=zsrc[:, :w])
        zero_fill(part1, PART1_ROWS)
        zero_fill(part2, PART1_ROWS)
        zero_fill(aggrc, AGGRC_ROWS)

        # ---------------- helpers ----------------
        def store_rows(dst, c, n, rm_tile):
            if n % 128 == 0:
                nc.gpsimd.dma_start(
                    out=dst[c:c + n, :].rearrange("(b p) d -> p b d", p=128),
                    in_=rm_tile[:, :n // 128, :])
            else:
                for b in range(0, n, 128):
                    rows = min(128, n - b)
                    nc.gpsimd.dma_start(out=dst[c + b:c + b + rows, :],
                                        in_=rm_tile[:rows, b // 128, :])

        def load_rows(dst_tile, src, c, n, p=128):
            if n % p == 0:
                nc.scalar.dma_start(out=dst_tile[:p, :n // p, :],
                                    in_=src[c:c + n, :].rearrange("(b p) d -> p b d", p=p))
            else:
                for b in range(0, n, p):
                    rows = min(p, n - b)
                    nc.scalar.dma_start(out=dst_tile[:rows, b // p, :],
                                        in_=src[c + b:c + b + rows, :])

        def tpose_cat(pcat, pofs, rm_tile, n, p=128):
            for b in range(0, n, p):
                rows = min(p, n - b)
                nc.tensor.transpose(out=pcat[pofs:pofs + H, b:b + rows],
                                    in_=rm_tile[:rows, b // p, :],
                                    identity=ident[:rows, :rows])

        def fm_to_rm(fm_ap, nblk, sbtag="rmsb"):
            """[64, nblk*128] feature-major -> row-major SBUF [128, nblk, 64]"""
            pt = ps_rm()
            for b in range(nblk):
                nc.tensor.transpose(out=pt[:, b, :], in_=fm_ap[:, b * 128:(b + 1) * 128],
                                    identity=ident[:H, :H])
            st = work.tile([128, 4, H], F32, tag=sbtag)
            nc.scalar.activation(out=st[:, :nblk, :], in_=pt[:, :nblk, :], func=AF.Copy)
            return st

        def mlp2_fm(rhs_ap, n, w1, b1, w2, b2):
            """feature-major 2-layer MLP: rhs [K,n] -> SBUF [64, n] (n<=512)."""
            p1 = ps_mm()
            nc.tensor.matmul(out=p1[:, :n], lhsT=w1[:], rhs=rhs_ap, start=True, stop=True)
            h1 = fmp.tile([H, 512], F32, tag="fm", bufs=6)
            nc.scalar.activation(out=h1[:, :n], in_=p1[:, :n], func=AF.Relu,
                                 bias=b1[:], scale=1.0)
            p2 = ps_mm()
            nc.tensor.matmul(out=p2[:, :n], lhsT=w2[:], rhs=h1[:, :n], start=True, stop=True)
            o = fmp.tile([H, 512], F32, tag="fm", bufs=6)
            nc.scalar.activation(out=o[:, :n], in_=p2[:, :n], func=AF.Copy,
                                 bias=b2[:], scale=1.0)
            return o

        def ln_relu_rm(src_ap, nblk, npart, g_tile, b_tile, residual_ap=None):
            """LayerNorm + ReLU (+residual) over row-major [npart, nblk, 64]
            (src may be PSUM). Returns SBUF tile (tag 'ln')."""
            mu = stats.tile([128, G], F32, tag="smu")
            nc.vector.tensor_reduce(out=mu[:npart, :nblk], in_=src_ap,
                                    axis=mybir.AxisListType.X, op=OP.add)
            nc.vector.tensor_scalar_mul(out=mu[:npart, :nblk], in0=mu[:npart, :nblk],
                                        scalar1=1.0 / H)
            zc = work.tile([128, 4, H], F32, tag="ln_zc")
            nc.vector.tensor_tensor(
                out=zc[:npart, :nblk, :], in0=src_ap,
                in1=mu[:npart, :nblk].rearrange("p n -> p n ()").to_broadcast([npart, nblk, H]),
                op=OP.subtract)
            sq = work.tile([128, 4, H], F32, tag="ln_sq")
            nc.scalar.activation(out=sq[:npart, :nblk, :], in_=zc[:npart, :nblk, :],
                                 func=AF.Square)
            var = stats.tile([128, G], F32, tag="svar")
            nc.vector.tensor_reduce(out=var[:npart, :nblk], in_=sq[:npart, :nblk, :],
                                    axis=mybir.AxisListType.X, op=OP.add)
            nc.scalar.activation(out=var[:npart, :nblk], in_=var[:npart, :nblk],
                                 func=AF.Sqrt, scale=1.0 / H, bias=epsc[:npart, :])
            nc.vector.reciprocal(out=var[:npart, :nblk], in_=var[:npart, :nblk])
            o = work.tile([128, 4, H], F32, tag="ln")
            nc.vector.tensor_tensor(
                out=o[:npart, :nblk, :], in0=zc[:npart, :nblk, :],
                in1=var[:npart, :nblk].rearrange("p n -> p n ()").to_broadcast([npart, nblk, H]),
                op=OP.mult)
            if g_tile is not None:
                nc.vector.tensor_tensor(
                    out=o[:npart, :nblk, :], in0=o[:npart, :nblk, :],
                    in1=g_tile[:npart, :].rearrange("p d -> p 1 d").to_broadcast([npart, nblk, H]),
                    op=OP.mult)
            if b_tile is not None:
                nc.vector.tensor_tensor(
                    out=o[:npart, :nblk, :], in0=o[:npart, :nblk, :],
                    in1=b_tile[:npart, :].rearrange("p d -> p 1 d").to_broadcast([npart, nblk, H]),
                    op=OP.add)
            nc.scalar.activation(out=o[:npart, :nblk, :], in_=o[:npart, :nblk, :], func=AF.Relu)
            if residual_ap is not None:
                nc.vector.tensor_tensor(out=o[:npart, :nblk, :], in0=o[:npart, :nblk, :],
                                        in1=residual_ap, op=OP.add)
            return o

        # ---------------- P1: parent encoder + pred1 table ----------------
        CH = 512
        for c in range(0, N_P, CH):
            n = min(CH, N_P - c)
            nblk = (n + 127) // 128
            rhs = fmp.tile([D_P, 512], F32, tag="encrhs")
            nc.scalar.dma_start(out=rhs[:, :n], in_=xpT[:, c:c + n])
            hpT = mlp2_fm(rhs[:, :n], n, W["wp1"], W["bp1"], W["wp2"], W["bp2"])
            predT = mlp2_fm(hpT[:, :n], n, W["l0_pw1"], W["l0_pb1"],
                            W["l0_pw2"], W["l0_pb2"])
            hp_rm = fm_to_rm(hpT[:], nblk)
            pr_rm = fm_to_rm(predT[:], nblk)
            store_rows(hp0, c, n, hp_rm)
            store_rows(tbl1, c, n, pr_rm)

        # ---------------- P2: child encoder ----------------
        for c in range(0, CSH, CH):
            n = min(CH, CSH - c)
            nblk = (n + 127) // 128
            rhs = fmp.tile([D_C, 512], F32, tag="encrhs")
            nc.scalar.dma_start(out=rhs[:, :n], in_=xcT[:, c:c + n])
            hcT = mlp2_fm(rhs[:, :n], n, W["wc1"], W["bc1"], W["wc2"], W["bc2"])
            hc_rm = fm_to_rm(hcT[:], nblk)
            store_rows(tbl1, N_P + c, n, hc_rm)

        # ---------------- edge message phase (layers 1 & 2) ----------------
        def msg_phase(tbl, part, tag):
            for s in range(NS1):
                gath = gathp.tile([128, 2 * G, H], F32, tag="gath")
                for g in range(2 * G):
                    nc.gpsimd.indirect_dma_start(
                        out=gath[:, g, :], out_offset=None, in_=tbl[:],
                        in_offset=bass.IndirectOffsetOnAxis(
                            ap=sb_gidx[:, s * 2 * G + g:s * 2 * G + g + 1], axis=0))
                z = work.tile([128, G, H], F32, tag="e_z")
                nc.vector.tensor_tensor(out=z[:], in0=gath[:, G:2 * G, :],
                                        in1=gath[:, 0:G, :], op=OP.subtract)
                mu = stats.tile([128, G], F32, tag="smu")
                nc.vector.tensor_reduce(out=mu[:], in_=z[:], axis=mybir.AxisListType.X,
                                        op=OP.add)
                nc.vector.tensor_scalar_mul(out=mu[:], in0=mu[:], scalar1=1.0 / H)
                zc = work.tile([128, G, H], F32, tag="e_zc")
                nc.vector.tensor_tensor(
                    out=zc[:], in0=z[:],
                    in1=mu[:].rearrange("p n -> p n ()").to_broadcast([128, G, H]),
                    op=OP.subtract)
                sq = work.tile([128, G, H], F32, tag="e_sq")
                nc.scalar.activation(out=sq[:], in_=zc[:], func=AF.Square)
                var = stats.tile([128, G], F32, tag="svar")
                nc.vector.tensor_reduce(out=var[:], in_=sq[:], axis=mybir.AxisListType.X,
                                        op=OP.add)
                nc.scalar.activation(out=var[:], in_=var[:], func=AF.Sqrt,
                                     scale=1.0 / H, bias=epsc[:])
                nc.vector.reciprocal(out=var[:], in_=var[:])
                nc.vector.tensor_mul(out=var[:], in0=var[:], in1=sb_w[:, s * G:(s + 1) * G])
                msgs = work.tile([128, G, H], F32, tag="e_msgs")
                nc.vector.tensor_tensor(
                    out=msgs[:], in0=zc[:],
                    in1=var[:].rearrange("p n -> p n ()").to_broadcast([128, G, H]),
                    op=OP.mult)
                selT = ps_sel()
                for g in range(G):
                    nc.tensor.transpose(
                        out=selT[:, g, :],
                        in_=sb_dstf[:, s * G + g:s * G + g + 1].to_broadcast([128, 128]),
                        identity=ident[:])
                sel = work.tile([128, G, 128], F32, tag="e_sel")
                nc.vector.tensor_tensor(
                    out=sel[:],
                    in0=sb_dstf[:, s * G:(s + 1) * G].rearrange("p n -> p n ()").to_broadcast([128, G, 128]),
                    in1=selT[:], op=OP.is_equal)
                dd = ps_dd()
                for g in range(G):
                    nc.tensor.matmul(out=dd[:, g, :], lhsT=sel[:, g, :],
                                     rhs=msgs[:, g, :], start=True, stop=True)
                msum = work.tile([128, G, H], BF16, tag="e_msumb")
                nc.scalar.activation(out=msum[:], in_=dd[:], func=AF.Copy)
                for g in range(G):
                    nc.gpsimd.indirect_dma_start(
                        out=part[:], out_offset=bass.IndirectOffsetOnAxis(
                            ap=sb_scat[:, s * G + g:s * G + g + 1], axis=0),
                        in_=msum[:, g, :], in_offset=None,
                        bounds_check=N_P - 1, oob_is_err=False)

        msg_phase(tbl1, part1, "m1")

        # ---------------- P4: AllReduce partial parent sums ----------------
        nc.gpsimd.collective_compute(
            "AllReduce", OP.add, replica_groups=[list(range(NCORES))],
            ins=[part1[0:N_P, :]], outs=[aggr1[:]])

        # ---------------- P5: child aggregation (layer 1) ----------------
        for s in range(NS2):
            gath = gathp.tile([128, 2 * G, H], F32, tag="gath")
            for g in range(G):
                nc.gpsimd.indirect_dma_start(
                    out=gath[:, g, :], out_offset=None, in_=hp0[:],
                    in_offset=bass.IndirectOffsetOnAxis(
                        ap=sb_cgidx[:, s * G + g:s * G + g + 1], axis=0))
            msgs = work.tile([128, G, H], F32, tag="e_msgs")
            nc.vector.tensor_tensor(
                out=msgs[:], in0=gath[:, 0:G, :],
                in1=sb_cw[:, s * G:(s + 1) * G].rearrange("p n -> p n ()").to_broadcast([128, G, H]),
                op=OP.mult)
            selT = ps_sel()
            for g in range(G):
                nc.tensor.transpose(
                    out=selT[:, g, :],
                    in_=sb_csrcf[:, s * G + g:s * G + g + 1].to_broadcast([128, 128]),
                    identity=ident[:])
            sel = work.tile([128, G, 128], F32, tag="e_sel")
            nc.vector.tensor_tensor(
                out=sel[:],
                in0=sb_csrcf[:, s * G:(s + 1) * G].rearrange("p n -> p n ()").to_broadcast([128, G, 128]),
                in1=selT[:], op=OP.is_equal)
            dd = ps_dd()
            for g in range(G):
                nc.tensor.matmul(out=dd[:, g, :], lhsT=sel[:, g, :], rhs=msgs[:, g, :],
                                 start=True, stop=True)
            msum = work.tile([128, G, H], F32, tag="e_msum")
            nc.scalar.activation(out=msum[:], in_=dd[:], func=AF.Copy)
            for g in range(G):
                nc.gpsimd.indirect_dma_start(
                    out=aggrc[:], out_offset=bass.IndirectOffsetOnAxis(
                        ap=sb_cscat[:, s * G + g:s * G + g + 1], axis=0),
                    in_=msum[:, g, :], in_offset=None,
                    bounds_check=CSH - 1, oob_is_err=False)

        # ---------------- P6: layer-1 update (replicated) + pred2 ----------
        for c in range(0, N_P, CH):
            n = min(CH, N_P - c)
            nblk = (n + 127) // 128
            hp0rm = work.tile([128, 4, H], F32, tag="nrm_a")
            load_rows(hp0rm, hp0, c, n)
            ag1rm_b = work.tile([128, 4, H], BF16, tag="nrm_bb")
            load_rows(ag1rm_b, aggr1, c, n)
            ag1rm = work.tile([128, 4, H], F32, tag="nrm_b")
            nc.vector.tensor_copy(out=ag1rm[:], in_=ag1rm_b[:])
            pcat = ps_cat()
            tpose_cat(pcat, 0, hp0rm, n)
            tpose_cat(pcat, H, ag1rm, n)
            rhs = fmp.tile([2 * H, 512], F32, tag="catrhs")
            nc.scalar.activation(out=rhs[:, :n], in_=pcat[:, :n], func=AF.Copy)
            pu = ps_mm()
            nc.tensor.matmul(out=pu[:, :n], lhsT=W["l0_updw"][:], rhs=rhs[:, :n],
                             start=True, stop=True)
            uT = fmp.tile([H, 512], F32, tag="fm", bufs=6)
            nc.scalar.activation(out=uT[:, :n], in_=pu[:, :n], func=AF.Copy,
                                 bias=W["l0_updb"][:], scale=1.0)
            urm = ps_rm()
            for b in range(nblk):
                nc.tensor.transpose(out=urm[:, b, :], in_=uT[:, b * 128:(b + 1) * 128],
                                    identity=ident[:H, :H])
            hp1_rm = ln_relu_rm(urm[:, :nblk, :], nblk, 128,
                                lnbc["l0_lnpg"], lnbc["l0_lnpb"])
            store_rows(hp1, c, n, hp1_rm)
            ph1T = ps_cat()
            tpose_cat(ph1T, 0, hp1_rm, n)
            h1T = fmp.tile([H, 512], F32, tag="h1T")
            nc.scalar.activation(out=h1T[:, :n], in_=ph1T[0:H, :n], func=AF.Copy)
            predT = mlp2_fm(h1T[:, :n], n, W["l1_pw1"], W["l1_pb1"],
                            W["l1_pw2"], W["l1_pb2"])
            pr_rm = fm_to_rm(predT[:], nblk)
            store_rows(tbl2, c, n, pr_rm)

        # ---------------- P7: SAGE child update (shard) ----------------
        for c in range(0, CSH, CH):
            n = min(CH, CSH - c)
            nblk = (n + 127) // 128
            acrm = work.tile([128, 4, H], F32, tag="nrm_a")
            load_rows(acrm, aggrc, c, n)
            hc0rm = work.tile([128, 4, H], F32, tag="nrm_b")
            load_rows(hc0rm, tbl1, N_P + c, n)
            pcat = ps_cat()
            tpose_cat(pcat, 0, acrm, n)
            tpose_cat(pcat, H, hc0rm, n)
            rhs = fmp.tile([2 * H, 512], F32, tag="catrhs")
            nc.scalar.activation(out=rhs[:, :n], in_=pcat[:, :n], func=AF.Copy)
            pu = ps_mm()
            nc.tensor.matmul(out=pu[:, :n], lhsT=W["l0_sagew"][:], rhs=rhs[:, :n],
                             start=True, stop=True)
            uT = fmp.tile([H, 512], F32, tag="fm", bufs=6)
            nc.scalar.activation(out=uT[:, :n], in_=pu[:, :n], func=AF.Copy,
                                 bias=W["l0_sageb"][:], scale=1.0)
            urm = ps_rm()
            for b in range(nblk):
                nc.tensor.transpose(out=urm[:, b, :], in_=uT[:, b * 128:(b + 1) * 128],
                                    identity=ident[:H, :H])
            hc1_rm = ln_relu_rm(urm[:, :nblk, :], nblk, 128,
                                lnbc["l0_lncg"], lnbc["l0_lncb"])
            store_rows(tbl2, N_P + c, n, hc1_rm)

        # ---------------- P8: layer-2 message phase ----------------
        msg_phase(tbl2, part2, "m2")

        # ---------------- P9: ReduceScatter ----------------
        nc.gpsimd.collective_compute(
            "ReduceScatter", OP.add, replica_groups=[list(range(NCORES))],
            ins=[part2[0:N_P, :]], outs=[rs2[:]])

        # ---------------- P10: layer-2 tail + head (shard) ----------------
        pid = nc.gpsimd.partition_id()
        CH2 = 500
        PP = 125
        for c in range(0, PSH, CH2):
            n = CH2
            hp1_rm = work.tile([128, 4, H], F32, tag="p10_hp1rm")
            nc.gpsimd.dma_start(
                out=hp1_rm[:PP, :, :],
                in_=hp1[bass.ds(pid * PSH + c, n), :].rearrange("(b p) d -> p b d", p=PP))
            rs2rm_b = work.tile([128, 4, H], BF16, tag="nrm_bb")
            nc.scalar.dma_start(out=rs2rm_b[:PP, :, :],
                                in_=rs2[c:c + n, :].rearrange("(b p) d -> p b d", p=PP))
            rs2rm = work.tile([128, 4, H], F32, tag="nrm_b")
            nc.vector.tensor_copy(out=rs2rm[:PP, :, :], in_=rs2rm_b[:PP, :, :])
            pcat = ps_cat()
            tpose_cat(pcat, 0, hp1_rm, n, p=PP)
            tpose_cat(pcat, H, rs2rm, n, p=PP)
            rhs = fmp.tile([2 * H, 512], F32, tag="catrhs")
            nc.scalar.activation(out=rhs[:, :n], in_=pcat[:, :n], func=AF.Copy)
            pu = ps_mm()
            nc.tensor.matmul(out=pu[:, :n], lhsT=W["l1_updw"][:], rhs=rhs[:, :n],
                             start=True, stop=True)
            uT = fmp.tile([H, 512], F32, tag="fm", bufs=6)
            nc.scalar.activation(out=uT[:, :n], in_=pu[:, :n], func=AF.Copy,
                                 bias=W["l1_updb"][:], scale=1.0)
            urm = ps_rm()
            for b in range(4):
                nc.tensor.transpose(out=urm[:PP, b, :], in_=uT[:, b * PP:(b + 1) * PP],
                                    identity=ident[:H, :H])
            hp2_rm = ln_relu_rm(urm[:PP, :4, :], 4, PP,
                                lnbc["l1_lnpg"], lnbc["l1_lnpb"],
                                residual_ap=hp1_rm[:PP, :, :])
            hfm = ps_mm()
            for b in range(4):
                nc.tensor.transpose(out=hfm[:, b * PP:(b + 1) * PP],
                                    in_=hp2_rm[:PP, b, :], identity=ident[:PP, :PP])
            hfms = fmp.tile([H, 512], F32, tag="fm", bufs=6)
            nc.scalar.activation(out=hfms[:, :n], in_=hfm[:, :n], func=AF.Copy)
            ph1 = ps_mm()
            nc.tensor.matmul(out=ph1[:H // 2, :n], lhsT=W["hw1"][:], rhs=hfms[:, :n],
                             start=True, stop=True)
            h1 = fmp.tile([H // 2, 512], F32, tag="headh1")
            nc.scalar.activation(out=h1[:, :n], in_=ph1[:H // 2, :n], func=AF.Relu,
                                 bias=W["hb1"][:], scale=1.0)
            po = ps_mm()
            nc.tensor.matmul(out=po[:1, :n], lhsT=W["hw2"][:], rhs=h1[:, :n],
                             start=True, stop=True)
            orow = fmp.tile([1, 512], F32, tag="orow")
            nc.scalar.activation(out=orow[:, :n], in_=po[:1, :n], func=AF.Copy,
                                 bias=W["hb2"][:], scale=1.0)
            nc.gpsimd.dma_start(out=out[0:1, c:c + n], in_=orow[:, :n])

        if DEBUG:
            for nm, t in (("hp0", hp0), ("tbl1", tbl1), ("tbl2", tbl2),
                          ("hp1", hp1), ("part1", part1), ("aggrc", aggrc),
                          ("aggr1", aggr1)):
                dt_ = nc.dram_tensor("dbg_" + nm, list(t.shape), t.dtype,
                                     kind="ExternalOutput")
                nc.gpsimd.dma_start(out=dt_[:, :], in_=t[:, :])

    nc.compile()
    return nc


# ----------------------------------------------------------------------------
# Entry point
# ----------------------------------------------------------------------------

_CACHE = {}


def kernel(x_parent, x_child, edge_index, params):
    x_parent = np.asarray(x_parent, np.float32)
    x_child = np.asarray(x_child, np.float32)
    edge_index = np.asarray(edge_index)
    Pw = _fold_params(params)
    metas, nb1, nb2 = _prep_edges(edge_index[0], edge_index[1])

    ln_identity = {}
    for nm in ("l0_lnpg", "l0_lncg", "l1_lnpg"):
        ln_identity[nm] = bool(np.all(Pw[nm] == 1.0))
    for nm in ("l0_lnpb", "l0_lncb", "l1_lnpb"):
        ln_identity[nm] = bool(np.all(Pw[nm] == 0.0))

    key = (nb1, nb2, tuple(sorted(ln_identity.items())))
    if key not in _CACHE:
        _CACHE[key] = build_program(Pw, nb1, nb2, ln_identity)
    nc = _CACHE[key]

    xpT = np.ascontiguousarray(x_parent.T)
    in_maps = []
    for k in range(NCORES):
        im = {"xpT": xpT,
              "xcT": np.ascontiguousarray(x_child[k * CSH:(k + 1) * CSH].T)}
        im.update(Pw)
        im.update(metas[k])
        in_maps.append(im)

    res = run_bass_kernel_spmd(nc, in_maps, core_ids=list(range(NCORES)))
    return np.concatenate([res.results[k]["out"][0] for k in range(NCORES)]).astype(np.float32)
